# revision 1
# baseline (speedup 1.0000x reference)
"""Trainium2 Bass kernel for nn_BatchGraphEncoder (gnn_message_passing).

Math note: the reference's segment softmax uses B unique segment ids
(groups of size 1), so alpha == exp(x-x)/1 == 1.0 bit-exactly for any
finite scores.  The output is therefore independent of the attention
inputs (w_i, w_j, w_k) and reduces to pure batch sums:

    out[:,   0:128] = sum_b h[b,:]      (broadcast over the N=512 rows)
    out[:, 128:256] = sum_b r[b,:]      (broadcast)
    out[:, 256:384] = sum_b t[b,:,:]    ([512, 128])

This is a memory-bound reduction over B=2048 dominated by reading t
(512 MB).  Strategy: shard B across the 8 cores (data parallel), reduce
over the local batch on-device, and sum the 8 tiny partials on the host.

Per-core reduction runs on the VectorEngine (fp32 matmul on the PE is
~4x derated — LOW/HIGH double pass — so the DVE's 1 raw elem/cycle/lane
fold-adds are faster and hide fully under the ~360 GB/s DMA stream).
Tile layout: partition p holds flat columns [512p, 512p+512) of the
[B_loc, 65536] shard; the free dim packs NB batch rows.  In-place
halving folds reduce each tile into a width-1024 accumulator.  DMA
issue alternates between the SP and ACT HWDGE rings to parallelize
descriptor generation (2 KB runs -> ~34 K descriptors).

The h/r sums ride on the otherwise-idle TensorEngine: a stationary
matrix whose column j is all-ones places column-sums of the moving
operand into PSUM row j (rows 0/1 = sum_h/sum_r).

Load balancing: core 6 of this machine usually loses ~10% DMA
bandwidth (slow SDMA engine E15), so it gets a smaller shard: rows
[228, 260) of each shard are only loaded when partition_id != 6
(core 6's buffer holds zero padding there).  The two conditional tiles
sit mid-stream; their accumulator merge is gated by a per-partition
scalar mask so skipped-DMA garbage never reaches the sums, and h/r
padding rows are zeros, which is exact for a sum.
"""

import numpy as np

B, N, D = 2048, 512, 128
NCORES = 8
FLAT = N * D                 # 65536 flattened (n, d) columns
MMW = 512                    # columns per partition / fold unit

# Cores 6 and 4 of this machine intermittently lose ~10% DMA bandwidth
# (one slow SDMA engine each; core 6 most often, core 4 frequently), so
# they get smaller shards.  Rows [224, 240) are skipped on core 6, rows
# [240, 264) on cores 4 and 6 (their buffers hold zero padding there).
B_FAST = 264
SIZES = [B_FAST] * NCORES
SIZES[4] = 240
SIZES[6] = 224
assert sum(SIZES) == B

# (row0, NB, conditional) in emission order: 5 unconditional big tiles
# first (so conditional tiles never fold first-use SBUF on the skipping
# core), conditional tiles mid-stream (their garbage folds overlap the
# DMA stream there), and a RUN of small tiles at the end: a 4 MB tile's
# folds take ~9 us after its DMA lands, so finishing with 1 MB/0.5 MB
# tiles (fold ~2.3/1.2 us < their DMA time) drains the fold backlog
# before the stream ends, cutting the post-stream tail to ~2 us.
_A = [(r, 16, None) for r in range(0, 208, 16)]      # rows [0, 208) 13 big
_BT = [(208, 4, None), (212, 4, None), (216, 4, None), (220, 2, None), (222, 2, None)]
_C = [(224, 16, "c6"), (240, 16, "c46"), (256, 8, "c46")]  # rows [224, 264)
TILE_PLAN = _A[0:5] + [_C[0]] + _A[5:8] + [_C[1]] + _A[8:11] + [_C[2]] + _A[11:13] + _BT
assert sorted(r for r, nb, c in TILE_PLAN) == sorted(
    r for r, nb, c in _A + _BT + _C
)
assert sum(nb for _, nb, _ in TILE_PLAN) == B_FAST

_BUILT = None
# test.py can inject {"trace": True, ...} here; harness path leaves it empty.
RUN_KWARGS = {}
LAST_RESULTS = None


def _build():
    from concourse import bacc, tile, mybir

    f32 = mybir.dt.float32
    add = mybir.AluOpType.add
    nc = bacc.Bacc(
        "TRN2",
        target_bir_lowering=False,
        debug=False,
        enable_asserts=False,
        num_devices=NCORES,
    )
    t_in = nc.dram_tensor("t_shard", [B_FAST, FLAT], f32, kind="ExternalInput").ap()
    h_in = nc.dram_tensor("h_shard", [B_FAST, D], f32, kind="ExternalInput").ap()
    r_in = nc.dram_tensor("r_shard", [B_FAST, D], f32, kind="ExternalInput").ap()
    out_t = nc.dram_tensor("out_t_part", [128, MMW], f32, kind="ExternalOutput").ap()
    out_hr = nc.dram_tensor("out_hr_part", [2, D], f32, kind="ExternalOutput").ap()

    with tile.TileContext(nc) as tc:
        with (
            tc.tile_pool(name="wconst", bufs=1) as wpool,
            tc.tile_pool(name="loads", bufs=5) as loads,
            tc.tile_pool(name="hr", bufs=6) as hrpool,
            tc.tile_pool(name="res", bufs=1) as res,
            tc.tile_pool(name="acc", bufs=1, space="PSUM") as ppool,
        ):
            W = wpool.tile([128, 256], f32)
            mask6 = wpool.tile([128, 1], f32)
            mask46 = wpool.tile([128, 1], f32)
            psum_hr = ppool.tile([128, D], f32)
            acc = res.tile([128, 1024], f32)
            skip_cond = {}
            masks = {"c6": mask6, "c46": mask46}

            def emit_setup_and_hr():
                # Emitted after the first few t loads so the pid register
                # loads and h/r DMAs never delay the t stream's start; h/r
                # loads ride the SWDGE (gpsimd) ring, keeping both HWDGE
                # rings exclusively on t tiles.
                # W is zero except column 128 == 1.0; W[:, 128-j : 256-j]
                # is a [128, 128] stationary whose column j is all-ones.
                nc.vector.memset(W[:], 0.0)
                nc.vector.memset(W[:, 128:129], 1.0)
                # mask6/mask46 = 0.0 on the core(s) that skip that tier,
                # 1.0 elsewhere; they gate the accumulator merges of the
                # conditional tiles.
                nc.vector.memset(mask6[:], 1.0)
                nc.vector.memset(mask46[:], 1.0)
                pid_vec = nc.vector.partition_id()
                with tc.If(pid_vec == 6):
                    nc.vector.memset(mask6[:], 0.0)
                    nc.vector.memset(mask46[:], 0.0)
                with tc.If(pid_vec == 4):
                    nc.vector.memset(mask46[:], 0.0)
                pid_sync = nc.sync.partition_id()
                pid_act = nc.scalar.partition_id()
                skip_cond["c6"] = {
                    nc.sync: pid_sync != 6,
                    nc.scalar: pid_act != 6,
                }
                skip_cond["c46"] = {
                    nc.sync: (pid_sync != 6) * (pid_sync != 4),
                    nc.scalar: (pid_act != 6) * (pid_act != 4),
                }

                # h / r batch sums -> rows 0 / 1 of psum_hr
                # (core 6's padding rows are zeros; adding them is exact)
                chunks = []
                for row, src in ((0, h_in), (1, r_in)):
                    for c0 in range(0, B_FAST, 128):
                        k = min(128, B_FAST - c0)
                        ht = hrpool.tile([128, D], f32)
                        nc.gpsimd.dma_start(ht[:k, :], src[c0 : c0 + k, :])
                        chunks.append((row, ht, k))
                for i, (row, ht, k) in enumerate(chunks):
                    nc.tensor.matmul(
                        psum_hr[:],
                        W[:k, 128 - row : 256 - row],
                        ht[:k, :],
                        start=(i == 0),
                        stop=(i == len(chunks) - 1),
                    )

            # --- t batch sum on the DVE ---
            for k, (b0, NB, cnd) in enumerate(TILE_PLAN):
                if k == 3:
                    emit_setup_and_hr()
                fw = NB * MMW  # free width
                tl = loads.tile([128, 16 * MMW], f32)
                src = t_in[b0 : b0 + NB, :].rearrange("b (p c) -> p b c", p=128)
                dma = nc.sync if k % 2 == 0 else nc.scalar
                dst = tl[:, :fw].rearrange("p (b c) -> p b c", b=NB)
                if cnd:
                    # Skipped on the slow core(s): the slot then holds stale
                    # (finite) data from an earlier tile; the masked merge
                    # zeroes it.
                    dma.dma_start(dst, src, cond=skip_cond[cnd][dma])
                else:
                    dma.dma_start(dst, src)
                half = fw // 2
                while half >= 1024:
                    nc.vector.tensor_tensor(
                        tl[:, :half], tl[:, :half], tl[:, half : 2 * half], add
                    )
                    half //= 2
                if k == 0:
                    nc.vector.tensor_copy(acc[:], tl[:, :1024])
                elif cnd:
                    # acc = (tile_fold * mask) + acc
                    nc.vector.scalar_tensor_tensor(
                        acc[:],
                        tl[:, :1024],
                        masks[cnd][:],
                        acc[:],
                        mybir.AluOpType.mult,
                        add,
                    )
                else:
                    nc.vector.tensor_tensor(acc[:], acc[:], tl[:, :1024], add)

            res_t = res.tile([128, MMW], f32)
            nc.vector.tensor_tensor(res_t[:], acc[:, :512], acc[:, 512:], add)
            nc.sync.dma_start(out_t[:], res_t[:])

            res_hr = res.tile([2, D], f32)
            nc.vector.tensor_copy(res_hr[:], psum_hr[0:2, :])
            nc.sync.dma_start(out_hr[:], res_hr[:])

    nc.compile()
    return nc


def _get_built():
    global _BUILT
    if _BUILT is None:
        _BUILT = _build()
    return _BUILT


def kernel(h, r, t, w_i, w_j, w_k):
    global LAST_RESULTS
    from concourse import bass_utils

    nc = _get_built()
    t2 = np.ascontiguousarray(t, dtype=np.float32).reshape(B, FLAT)
    h = np.ascontiguousarray(h, dtype=np.float32)
    r = np.ascontiguousarray(r, dtype=np.float32)

    def pad(a, ncols):
        out = np.zeros((B_FAST, ncols), dtype=np.float32)
        out[: a.shape[0]] = a
        return out

    starts = np.concatenate([[0], np.cumsum(SIZES)])
    in_maps = []
    for c in range(NCORES):
        s, e = int(starts[c]), int(starts[c + 1])
        if e - s == B_FAST:
            in_maps.append({"t_shard": t2[s:e], "h_shard": h[s:e], "r_shard": r[s:e]})
        else:
            in_maps.append(
                {
                    "t_shard": pad(t2[s:e], FLAT),
                    "h_shard": pad(h[s:e], D),
                    "r_shard": pad(r[s:e], D),
                }
            )
    results = bass_utils.run_bass_kernel_spmd(
        nc, in_maps, core_ids=list(range(NCORES)), **RUN_KWARGS
    )
    LAST_RESULTS = results

    sum_t = np.zeros(FLAT, dtype=np.float64)
    sum_h = np.zeros(D, dtype=np.float64)
    sum_r = np.zeros(D, dtype=np.float64)
    for c in range(NCORES):
        sum_t += results.results[c]["out_t_part"].reshape(FLAT)
        sum_h += results.results[c]["out_hr_part"][0]
        sum_r += results.results[c]["out_hr_part"][1]

    out = np.empty((N, 3 * D), dtype=np.float32)
    out[:, 0:D] = sum_h.astype(np.float32)[None, :]
    out[:, D : 2 * D] = sum_r.astype(np.float32)[None, :]
    out[:, 2 * D :] = sum_t.astype(np.float32).reshape(N, D)
    return out



# revision 2
# speedup vs baseline: 1.0950x; 1.0950x over previous
"""Trainium2 Bass kernel for nn_BatchGraphEncoder (gnn_message_passing).

Math note: the reference's segment softmax uses B unique segment ids
(groups of size 1), so alpha == exp(x-x)/1 == 1.0 bit-exactly for any
finite scores.  The output is therefore independent of the attention
inputs (w_i, w_j, w_k) and reduces to pure batch sums:

    out[:,   0:128] = sum_b h[b,:]      (broadcast over the N=512 rows)
    out[:, 128:256] = sum_b r[b,:]      (broadcast)
    out[:, 256:384] = sum_b t[b,:,:]    ([512, 128])

This is a memory-bound reduction over B=2048 dominated by reading t
(512 MB).  Strategy: shard B across the 8 cores (data parallel), reduce
over the local batch on-device, and sum the 8 tiny partials on the host.

Pipeline design (v2): the previous per-tile fold-tree (3 halving folds
+ accumulator merge, ~10.5us of DVE per 4 MB tile) slot-coupled the DMA
stream to the DVE and collapsed the last ~40us of the stream to fold
pace.  Now each tile gets ONE DVE op: tensor_tensor add into a wide
[128, 4096] accumulator (free dim packs 8 batch-row slots x 512 cols).
2 MB tiles / 10 pool buffers keep the DVE ~20% faster than the DMA
stream with a 20 MB elasticity window; a 1 MB-tile tail drains the
backlog, and one 4096->512 fold at the very end produces the partial.

Tile layout: partition p holds flat columns [512p, 512p+512) of the
[B_loc, 65536] shard; the free dim packs NB batch rows.  DMA issue
alternates between the SP and ACT HWDGE rings.

The h/r sums ride on the otherwise-idle TensorEngine: a stationary
matrix whose column j is all-ones places column-sums of the moving
operand into PSUM row j (rows 0/1 = sum_h/sum_r).

Load balancing: cores 4 and 6 of this machine usually lose ~10% DMA
bandwidth (one slow SDMA engine each), so they get smaller shards:
rows [224, 240) are only loaded when partition_id != 6, rows [240, 264)
when partition_id not in {4, 6} (the skipping cores' buffers hold stale
finite data there; a per-partition scalar mask gates those tiles'
accumulator merges, and h/r padding rows are zeros, which is exact).
"""

import numpy as np

B, N, D = 2048, 512, 128
NCORES = 8
FLAT = N * D                 # 65536 flattened (n, d) columns
MMW = 512                    # columns per row-slot in the free dim
ACCW = 8 * MMW               # accumulator free width (8 row slots)

# Cores 6 and 4 intermittently lose ~10% DMA bandwidth, so they get
# smaller shards.  Rows [224, 240) are skipped on core 6, rows
# [240, 264) on cores 4 and 6.
B_FAST = 264
SIZES = [B_FAST] * NCORES
SIZES[4] = 240
SIZES[6] = 224
assert sum(SIZES) == B

# (row0, NB, conditional) in emission order.  Conditional tiles sit
# mid-stream at slot indices >= bufs so they never touch first-use SBUF
# on a skipping core.  The tail is 4 x 1 MB tiles: their 2.3us merges
# sit below their 2.8us DMA time, so the merge backlog drains with the
# stream instead of after it.
_U = [(r, 8, None) for r in range(0, 208, 8)]        # 26 big tiles
_C6 = [(224, 8, "c6"), (232, 8, "c6")]
_C46 = [(240, 8, "c46"), (248, 8, "c46"), (256, 8, "c46")]
_TAIL = [(r, 4, None) for r in range(208, 224, 4)]   # 4 small tiles
TILE_PLAN = (
    _U[0:12]
    + [_C6[0]]
    + _U[12:15]
    + [_C6[1]]
    + _U[15:18]
    + [_C46[0]]
    + _U[18:21]
    + [_C46[1]]
    + _U[21:24]
    + [_C46[2]]
    + _U[24:26]
    + _TAIL
)
assert sum(nb for _, nb, _ in TILE_PLAN) == B_FAST
assert sorted(r for r, nb, c in TILE_PLAN) == sorted(
    r for r, nb, c in _U + _C6 + _C46 + _TAIL
)

NBUFS = 10

_BUILT = None
# test.py can inject {"trace": True, ...} here; harness path leaves it empty.
RUN_KWARGS = {}
LAST_RESULTS = None


def _build():
    from concourse import bacc, tile, mybir

    f32 = mybir.dt.float32
    add = mybir.AluOpType.add
    nc = bacc.Bacc(
        "TRN2",
        target_bir_lowering=False,
        debug=False,
        enable_asserts=False,
        num_devices=NCORES,
    )
    t_in = nc.dram_tensor("t_shard", [B_FAST, FLAT], f32, kind="ExternalInput").ap()
    h_in = nc.dram_tensor("h_shard", [B_FAST, D], f32, kind="ExternalInput").ap()
    r_in = nc.dram_tensor("r_shard", [B_FAST, D], f32, kind="ExternalInput").ap()
    out_t = nc.dram_tensor("out_t_part", [128, MMW], f32, kind="ExternalOutput").ap()
    out_hr = nc.dram_tensor("out_hr_part", [2, D], f32, kind="ExternalOutput").ap()

    with tile.TileContext(nc) as tc:
        with (
            tc.tile_pool(name="wconst", bufs=1) as wpool,
            tc.tile_pool(name="loads", bufs=NBUFS) as loads,
            tc.tile_pool(name="hr", bufs=6) as hrpool,
            tc.tile_pool(name="res", bufs=1) as res,
            tc.tile_pool(name="acc", bufs=1, space="PSUM") as ppool,
        ):
            W = wpool.tile([128, 256], f32)
            mask6 = wpool.tile([128, 1], f32)
            mask46 = wpool.tile([128, 1], f32)
            psum_hr = ppool.tile([128, D], f32)
            acc = res.tile([128, ACCW], f32)
            res_hr = res.tile([2, D], f32)
            skip_cond = {}
            masks = {"c6": mask6, "c46": mask46}

            def emit_setup_and_hr():
                # Emitted after the first few t loads so the pid register
                # loads and h/r DMAs never delay the t stream's start; h/r
                # loads ride the SWDGE (gpsimd) ring, keeping both HWDGE
                # rings exclusively on t tiles.
                # W is zero except column 128 == 1.0; W[:, 128-j : 256-j]
                # is a [128, 128] stationary whose column j is all-ones.
                nc.vector.memset(W[:], 0.0)
                nc.vector.memset(W[:, 128:129], 1.0)
                # mask6/mask46 = 0.0 on the core(s) that skip that tier,
                # 1.0 elsewhere; they gate the accumulator merges of the
                # conditional tiles.
                nc.vector.memset(mask6[:], 1.0)
                nc.vector.memset(mask46[:], 1.0)
                pid_vec = nc.vector.partition_id()
                with tc.If(pid_vec == 6):
                    nc.vector.memset(mask6[:], 0.0)
                    nc.vector.memset(mask46[:], 0.0)
                with tc.If(pid_vec == 4):
                    nc.vector.memset(mask46[:], 0.0)
                pid_sync = nc.sync.partition_id()
                pid_act = nc.scalar.partition_id()
                skip_cond["c6"] = {
                    nc.sync: pid_sync != 6,
                    nc.scalar: pid_act != 6,
                }
                skip_cond["c46"] = {
                    nc.sync: (pid_sync != 6) * (pid_sync != 4),
                    nc.scalar: (pid_act != 6) * (pid_act != 4),
                }

                # h / r batch sums -> rows 0 / 1 of psum_hr
                # (padding rows on short-shard cores are zeros; exact)
                chunks = []
                for row, src in ((0, h_in), (1, r_in)):
                    for c0 in range(0, B_FAST, 128):
                        k = min(128, B_FAST - c0)
                        ht = hrpool.tile([128, D], f32)
                        nc.gpsimd.dma_start(ht[:k, :], src[c0 : c0 + k, :])
                        chunks.append((row, ht, k))
                for i, (row, ht, k) in enumerate(chunks):
                    nc.tensor.matmul(
                        psum_hr[:],
                        W[:k, 128 - row : 256 - row],
                        ht[:k, :],
                        start=(i == 0),
                        stop=(i == len(chunks) - 1),
                    )
                # Ship the h/r partial mid-stream, off the tail.
                nc.vector.tensor_copy(res_hr[:], psum_hr[0:2, :])
                nc.sync.dma_start(out_hr[:], res_hr[:])

            # --- t batch sum: one DVE merge per tile into acc ---
            for k, (b0, NB, cnd) in enumerate(TILE_PLAN):
                if k == 2:
                    emit_setup_and_hr()
                fw = NB * MMW  # free width
                tl = loads.tile([128, 8 * MMW], f32)
                src = t_in[b0 : b0 + NB, :].rearrange("b (p c) -> p b c", p=128)
                dma = nc.sync if k % 2 == 0 else nc.scalar
                dst = tl[:, :fw].rearrange("p (b c) -> p b c", b=NB)
                if cnd:
                    # Skipped on the slow core(s): the slot then holds stale
                    # (finite) data from an earlier tile; the masked merge
                    # zeroes it.
                    dma.dma_start(dst, src, cond=skip_cond[cnd][dma])
                else:
                    dma.dma_start(dst, src)
                if k == 0:
                    nc.vector.tensor_copy(acc[:], tl[:, :ACCW])
                elif cnd:
                    # acc = (tile * mask) + acc
                    nc.vector.scalar_tensor_tensor(
                        acc[:, :fw],
                        tl[:, :fw],
                        masks[cnd][:],
                        acc[:, :fw],
                        mybir.AluOpType.mult,
                        add,
                    )
                else:
                    nc.vector.tensor_tensor(acc[:, :fw], acc[:, :fw], tl[:, :fw], add)

            # Final fold: 4096 -> 512 across the 8 row slots.
            half = ACCW // 2
            while half >= MMW:
                nc.vector.tensor_tensor(
                    acc[:, :half], acc[:, :half], acc[:, half : 2 * half], add
                )
                half //= 2
            nc.sync.dma_start(out_t[:], acc[:, :MMW])

    nc.compile()
    return nc


def _get_built():
    global _BUILT
    if _BUILT is None:
        _BUILT = _build()
    return _BUILT


def kernel(h, r, t, w_i, w_j, w_k):
    global LAST_RESULTS
    from concourse import bass_utils

    nc = _get_built()
    t2 = np.ascontiguousarray(t, dtype=np.float32).reshape(B, FLAT)
    h = np.ascontiguousarray(h, dtype=np.float32)
    r = np.ascontiguousarray(r, dtype=np.float32)

    def pad(a, ncols):
        out = np.zeros((B_FAST, ncols), dtype=np.float32)
        out[: a.shape[0]] = a
        return out

    starts = np.concatenate([[0], np.cumsum(SIZES)])
    in_maps = []
    for c in range(NCORES):
        s, e = int(starts[c]), int(starts[c + 1])
        if e - s == B_FAST:
            in_maps.append({"t_shard": t2[s:e], "h_shard": h[s:e], "r_shard": r[s:e]})
        else:
            in_maps.append(
                {
                    "t_shard": pad(t2[s:e], FLAT),
                    "h_shard": pad(h[s:e], D),
                    "r_shard": pad(r[s:e], D),
                }
            )
    results = bass_utils.run_bass_kernel_spmd(
        nc, in_maps, core_ids=list(range(NCORES)), **RUN_KWARGS
    )
    LAST_RESULTS = results

    sum_t = np.zeros(FLAT, dtype=np.float64)
    sum_h = np.zeros(D, dtype=np.float64)
    sum_r = np.zeros(D, dtype=np.float64)
    for c in range(NCORES):
        sum_t += results.results[c]["out_t_part"].reshape(FLAT)
        sum_h += results.results[c]["out_hr_part"][0]
        sum_r += results.results[c]["out_hr_part"][1]

    out = np.empty((N, 3 * D), dtype=np.float32)
    out[:, 0:D] = sum_h.astype(np.float32)[None, :]
    out[:, D : 2 * D] = sum_r.astype(np.float32)[None, :]
    out[:, 2 * D :] = sum_t.astype(np.float32).reshape(N, D)
    return out
